# revision 12
# baseline (speedup 1.0000x reference)
"""Trainium2 Bass kernel for nn_CandidateFinder (LSH hash-equality KNN).

Reference semantics: q/k binarized (x>0), projected by W [64,8], sign bits
packed into an 8-bit bucket code; for each query, return the first 64 key
indices (ascending) whose code equals the query's code, padded with -1.

Key insight: codes live in [0,256). Build, per batch, a [256, 64+] table of
the first-64 key indices per bucket, then gather per query via one-hot
matmuls. Sharding: 8 cores = 4 batches x 2 bucket-halves; host sums the
pair and subtracts 1 (tables store j+1, empty=0).

v2 layout/pipeline notes:
- k/q arrive packed [128, 1024]: partitions 0:64 carry D for j-chunks
  {0,2}-* no: cols 0:512 hold j 0:512 (parts 0:64) and j 512:1024
  (parts 64:128); cols 512:1024 hold j 1024:1536 / 1536:2048. One
  [128,512] binarize unlocks a full scan-half of hash matmuls.
- hash = two fp16 matmuls (W-hi + W-lo) accumulating f32 psum (sign
  margins ~1e-4; fp16 pair keeps representation error ~1e-6).
- k-side: relu half-0 split ACT||DVE to unblock the DVE scan sooner;
  scan half-1 runs on GPSIMD(Pool) chained off scan0's tail column.
- scatter idx via one copy_predicated into a preset -1 buffer
  (idx = 1-based rank at matches, -1 elsewhere).
- gather: 16 query-block matmuls x 2 half-tables accumulated in psum
  (no table merge); output DMA'd f32 straight from PSUM in quarters on
  parallel queues; host unpermutes the [128,16,64] block layout.
"""

import numpy as np
import ml_dtypes

B, L, D, NH = 4, 2048, 64, 8
KMAX = 64
TABLE_ELEMS = 1026  # local_scatter table size; 1-based ranks, slot 0 unused
MPAD = 40           # hash matmul lhsT free size: 8 real + 32 zero rows
HALF = L // 2
NBLK = 16           # query blocks of 128

_cache = {}


def _build_program():
    import concourse.bass as bass
    import concourse.mybir as mybir
    from concourse import bacc, tile
    from contextlib import ExitStack

    dt = mybir.dt
    Alu = mybir.AluOpType
    Act = mybir.ActivationFunctionType

    nc = bacc.Bacc("TRN2", target_bir_lowering=False, debug=False)

    # DRAM I/O (per-core shapes)
    kT_d = nc.declare_dram_parameter("kTp", [128, HALF], dt.bfloat16, isOutput=False)
    qT_d = nc.declare_dram_parameter("qTp", [128, HALF], dt.bfloat16, isOutput=False)
    wpk_d = nc.declare_dram_parameter("wpk", [128, 2 * MPAD], dt.float16, isOutput=False)
    sgnc_d = nc.declare_dram_parameter("sgnc", [128, 128], dt.float16, isOutput=False)
    out_d = nc.declare_dram_parameter("out", [128, NBLK * KMAX], dt.float16, isOutput=True)

    with ExitStack() as ctx:
        tc = ctx.enter_context(tile.TileContext(nc))
        sb = ctx.enter_context(tc.tile_pool(name="sb", bufs=1))
        hp = ctx.enter_context(tc.tile_pool(name="hp", bufs=2, space="PSUM"))
        aps = ctx.enter_context(tc.tile_pool(name="aps", bufs=2, space="PSUM"))
        gp = ctx.enter_context(tc.tile_pool(name="gp", bufs=2, space="PSUM"))

        from concourse.tile_rust import add_dep_helper  # noqa: E402

        # ---- input DMAs: k first (critical path), q split across queues ----
        kT_sb = sb.tile([128, HALF], dt.bfloat16, tag="kT")
        nc.sync.dma_start(kT_sb[:, 0:512], kT_d[:, 0:512])
        nc.sync.dma_start(kT_sb[:, 512:1024], kT_d[:, 512:1024])
        qT_sb = sb.tile([128, HALF], dt.bfloat16, tag="qT")
        nc.scalar.dma_start(qT_sb[:, 0:512], qT_d[:, 0:512])
        wpk_sb = sb.tile([128, 2 * MPAD], dt.float16, tag="wpk")
        nc.gpsimd.dma_start(wpk_sb[:], wpk_d[:])
        sgnc_sb = sb.tile([128, 128], dt.float16, tag="sgnc")
        nc.gpsimd.dma_start(sgnc_sb[:], sgnc_d[:])
        nc.gpsimd.dma_start(qT_sb[:, 512:1024], qT_d[:, 512:1024])

        # scatter values: each partition holds 1..L (fp16; ints <= 2048 exact)
        iota_sb = sb.tile([128, L], dt.float16, tag="iota")
        nc.gpsimd.iota(iota_sb[:], pattern=[[1, L]], base=1, channel_multiplier=0,
                       allow_small_or_imprecise_dtypes=True)

        # bias constant for Relu(agree - 7)
        bias7 = sb.tile([128, 1], dt.float32, tag="bias7")
        nc.gpsimd.memset(bias7[:], -7.0)

        # ---- PE warm-up: keep tensor engine busy until the p-state ramp
        # (full clock at t>3us) so real matmuls run at full speed ----
        warm_src = sb.tile([64, 512], dt.float16, tag="warm")
        nc.vector.memset(warm_src[:], 0.0)
        for r in range(5):
            wp = hp.tile([128, 512], dt.float32, tag="hp")
            nc.tensor.matmul(
                wp[:], lhsT=warm_src[:, 0:128], rhs=warm_src[:],
                start=True, stop=True,
            )
        warm_sink = sb.tile([1, 1], dt.float32, tag="warmsink")
        nc.vector.tensor_copy(warm_sink[:], wp[0:1, 0:1])

        # ---- binarize (DVE, {0,1} fp16) ----
        xk = sb.tile([128, HALF], dt.float16, tag="xk")
        xq = sb.tile([128, HALF], dt.float16, tag="xq")
        bin_ops = {}
        for tag, src, dst, a, b in (
            ("kA", kT_sb, xk, 0, 512),
            ("qA", qT_sb, xq, 0, 512),
            ("kB", kT_sb, xk, 512, 1024),
            ("qB", qT_sb, xq, 512, 1024),
        ):
            bin_ops[tag] = nc.vector.tensor_single_scalar(
                dst[:, a:b], src[:, a:b], 0.0, Alu.is_gt
            )

        # ---- k-side: hash -> sign -> agree -> onehot, per j-half ----
        # pair h covers j-chunks (2h, 2h+1): rhs partitions 0:64 / 64:128 of
        # packed cols 512h:512h+512. psum 2-stack at row bases {0 (M=40 pad),
        # 32 (M=8)}.
        oh_k = sb.tile([128, L], dt.float16, tag="ohk")
        rank = sb.tile([128, L], dt.float16, tag="rank")
        sk = []
        agree_k = []
        for h in range(2):
            c0, c1 = 512 * h, 512 * (h + 1)
            t = hp.tile([128, 512], dt.float32, tag="hp")
            nc.tensor.matmul(t[0:MPAD, :], lhsT=wpk_sb[0:64, 0:MPAD],
                             rhs=xk[0:64, c0:c1], start=True, stop=False)
            nc.tensor.matmul(t[0:MPAD, :], lhsT=wpk_sb[0:64, MPAD:2 * MPAD],
                             rhs=xk[0:64, c0:c1], start=False, stop=True)
            nc.tensor.matmul(t[32:40, :], lhsT=wpk_sb[64:128, 0:NH],
                             rhs=xk[64:128, c0:c1], start=True, stop=False)
            nc.tensor.matmul(t[32:40, :], lhsT=wpk_sb[64:128, MPAD:MPAD + NH],
                             rhs=xk[64:128, c0:c1], start=False, stop=True)
            s = sb.tile([MPAD, 512], dt.float16, tag=f"sk{h}")
            nc.scalar.activation(s[:], t[0:MPAD, :], Act.Sign)
            sk.append(s)
            apt = aps.tile([128, 1024], dt.float32, tag="agree")
            agree_k.append(
                nc.tensor.matmul(apt[:, 0:512], lhsT=sgnc_sb[0:8, :],
                                 rhs=s[0:8, :], start=True, stop=True)
            )
            agree_k.append(
                nc.tensor.matmul(apt[:, 512:1024], lhsT=sgnc_sb[32:40, :],
                                 rhs=s[32:40, :], start=True, stop=True)
            )
            lo = 1024 * h
            if h == 0:
                # split relu: ACT does the first 512 cols, DVE the second,
                # so scan half-0 starts as early as possible
                nc.scalar.activation(oh_k[:, lo:lo + 512], apt[:, 0:512],
                                     Act.Relu, bias=bias7[:])
                nc.vector.tensor_single_scalar(
                    oh_k[:, lo + 512:lo + 1024], apt[:, 512:1024], 7.0, Alu.is_gt
                )
            else:
                nc.scalar.activation(oh_k[:, lo:lo + 1024], apt[:],
                                     Act.Relu, bias=bias7[:])

        # ---- rank scan (DVE) + idx mask + scatter, pipelined per half:
        # scan0, m1/idx0 (scatter0 starts), scan1 (chained), m1/idx1 ----
        m1 = sb.tile([128, L], dt.float16, tag="m1")
        idx16 = sb.tile([128, L], dt.int16, tag="idx16")
        tabs = []
        for h in range(2):
            lo, hi = HALF * h, HALF * (h + 1)
            init = 0.0 if h == 0 else rank[:, HALF - 1:HALF]
            nc.vector.tensor_tensor_scan(
                rank[:, lo:hi], oh_k[:, lo:hi], oh_k[:, lo:hi],
                init, Alu.add, Alu.bypass,
            )
            nc.vector.tensor_mul(m1[:, lo:hi], oh_k[:, lo:hi], rank[:, lo:hi])
            nc.vector.tensor_single_scalar(
                idx16[:, lo:hi], m1[:, lo:hi], 1.0, Alu.subtract
            )
            tab = sb.tile([128, TABLE_ELEMS], dt.float16, tag=f"table{h}")
            tabs.append(tab)
            nc.gpsimd.local_scatter(
                tab[:], iota_sb[:, lo:hi], idx16[:, lo:hi],
                channels=128, num_elems=TABLE_ELEMS, num_idxs=HALF,
            )

        # ---- q-side: hash 2-stack pairs -> sign -> agree -> relu 3-way ----
        q1h = sb.tile([128, L], dt.float16, tag="q1h")
        aq = []
        for h in range(2):
            c0, c1 = 512 * h, 512 * (h + 1)
            t = hp.tile([128, 512], dt.float32, tag="hp")
            nc.tensor.matmul(t[0:MPAD, :], lhsT=wpk_sb[0:64, 0:MPAD],
                             rhs=xq[0:64, c0:c1], start=True, stop=False)
            nc.tensor.matmul(t[0:MPAD, :], lhsT=wpk_sb[0:64, MPAD:2 * MPAD],
                             rhs=xq[0:64, c0:c1], start=False, stop=True)
            nc.tensor.matmul(t[32:40, :], lhsT=wpk_sb[64:128, 0:NH],
                             rhs=xq[64:128, c0:c1], start=True, stop=False)
            nc.tensor.matmul(t[32:40, :], lhsT=wpk_sb[64:128, MPAD:MPAD + NH],
                             rhs=xq[64:128, c0:c1], start=False, stop=True)
            s = sb.tile([MPAD, 512], dt.float16, tag=f"sq{h}")
            nc.scalar.activation(s[:], t[0:MPAD, :], Act.Sign)
            apt = aps.tile([128, 1024], dt.float32, tag="agree")
            aq.append(apt)
            nc.tensor.matmul(apt[:, 0:512], lhsT=sgnc_sb[0:8, :],
                             rhs=s[0:8, :], start=True, stop=True)
            nc.tensor.matmul(apt[:, 512:1024], lhsT=sgnc_sb[32:40, :],
                             rhs=s[32:40, :], start=True, stop=True)
        # relu_q 3-way: ACT [0:1024], DVE [1024:1536], ACT [1536:2048]
        nc.scalar.activation(q1h[:, 0:1024], aq[0][:], Act.Relu, bias=bias7[:])
        nc.vector.tensor_single_scalar(q1h[:, 1024:1536], aq[1][:, 0:512],
                                       7.0, Alu.is_gt)
        nc.scalar.activation(q1h[:, 1536:2048], aq[1][:, 512:1024],
                             Act.Relu, bias=bias7[:])

        # ---- gather: out[128t+p, s] = sum_c q1h[c,128t+p] * tab[c, 1+s];
        # two half-tables accumulate in psum (no merge op) ----
        gps = []
        for gi in range(2):
            gpt = gp.tile([128, 8 * KMAX], dt.float32, tag="gp", name=f"gp{gi}")
            gps.append(gpt)
        for blk in range(NBLK):
            op = gps[blk // 8]
            cc = KMAX * (blk % 8)
            nc.tensor.matmul(op[:, cc:cc + KMAX],
                             lhsT=q1h[:, 128 * blk:128 * (blk + 1)],
                             rhs=tabs[0][:, 0:KMAX], start=True, stop=False)
            nc.tensor.matmul(op[:, cc:cc + KMAX],
                             lhsT=q1h[:, 128 * blk:128 * (blk + 1)],
                             rhs=tabs[1][:, 0:KMAX], start=False, stop=True)

        # ---- out: psum -> fp16 SBUF in quarters (ACT || DVE), then
        # quarter-DMAs on parallel queues ----
        QW = 4 * KMAX
        out_sb = sb.tile([128, NBLK * KMAX], dt.float16, tag="out_sb")
        for qi in range(4):
            src = gps[qi // 2][:, QW * (qi % 2):QW * (qi % 2 + 1)]
            dst = out_sb[:, QW * qi:QW * (qi + 1)]
            if qi % 2 == 0:
                nc.scalar.activation(dst, src, Act.Copy)
            else:
                nc.vector.tensor_copy(dst, src)
        nc.sync.dma_start(out_d[:, 0:QW], out_sb[:, 0:QW])
        nc.scalar.dma_start(out_d[:, QW:2 * QW], out_sb[:, QW:2 * QW])
        nc.gpsimd.dma_start(out_d[:, 2 * QW:3 * QW], out_sb[:, 2 * QW:3 * QW])
        nc.sync.dma_start(out_d[:, 3 * QW:4 * QW], out_sb[:, 3 * QW:4 * QW])

    nc.compile()
    return nc


def _get_nc():
    if "nc" not in _cache:
        _cache["nc"] = _build_program()
    return _cache["nc"]


def _pack(xT):
    # [D, L] -> [128, L/2]: chunk c (512 cols) of j goes to
    # (partitions 64*(c%2) .., cols 512*(c//2) ..)
    x4 = xT.reshape(D, 4, 512)
    out = np.empty((128, HALF), xT.dtype)
    out[0:64, 0:512] = x4[:, 0]
    out[64:128, 0:512] = x4[:, 1]
    out[0:64, 512:1024] = x4[:, 2]
    out[64:128, 512:1024] = x4[:, 3]
    return out


def _make_in_maps(query, key, W):
    query = np.asarray(query, dtype=np.float32)
    key = np.asarray(key, dtype=np.float32)
    W = np.asarray(W, dtype=np.float32)
    qTp = [
        _pack(np.ascontiguousarray(query[b].T).astype(ml_dtypes.bfloat16))
        for b in range(B)
    ]
    kTp = [
        _pack(np.ascontiguousarray(key[b].T).astype(ml_dtypes.bfloat16))
        for b in range(B)
    ]

    wpk = np.zeros((128, 2 * MPAD), np.float16)
    wpk[0:64, :NH] = W.astype(np.float16)
    wpk[0:64, MPAD:MPAD + NH] = (W - wpk[0:64, :NH].astype(np.float32)).astype(
        np.float16
    )
    wpk[64:128] = wpk[0:64]

    sgnc = []
    for h in range(2):
        cg = 128 * h + np.arange(128)  # global bucket ids of this half
        bits = ((cg[None, :] >> np.arange(NH)[:, None]) & 1).astype(np.float32)
        pm = (2.0 * bits - 1.0).astype(np.float16)  # [8, 128]
        arr = np.zeros((128, 128), np.float16)
        for base in (0, 32, 64, 96):
            arr[base:base + NH] = pm
        sgnc.append(arr)
    return [
        {"qTp": qTp[c // 2], "kTp": kTp[c // 2], "wpk": wpk, "sgnc": sgnc[c % 2]}
        for c in range(2 * B)
    ]


def _combine(results):
    out = np.empty((B, L, KMAX), dtype=np.int64)
    for b in range(B):
        g = np.zeros((L, KMAX), np.float64)
        for h in range(2):
            r = results[2 * b + h]["out"].astype(np.float64)
            g += r.reshape(128, NBLK, KMAX).transpose(1, 0, 2).reshape(L, KMAX)
        out[b] = (g - 1.0).astype(np.int64)
    return out


def _run_spmd(in_maps, **kwargs):
    from concourse.bass_utils import run_bass_kernel_spmd

    return run_bass_kernel_spmd(_get_nc(), in_maps, list(range(2 * B)), **kwargs)


def kernel(query, key, W, head_idx=0, **_unused):
    in_maps = _make_in_maps(query, key, W)
    res = _run_spmd(in_maps)
    return _combine(res.results)


# revision 19
# speedup vs baseline: 1.0593x; 1.0593x over previous
"""Trainium2 Bass kernel for nn_CandidateFinder (LSH hash-equality KNN).

Reference semantics: q/k binarized (x>0), projected by W [64,8], sign bits
packed into an 8-bit bucket code; for each query, return the first 64 key
indices (ascending) whose code equals the query's code, padded with -1.

Key insight: codes live in [0,256). Build, per batch, a [256, 64+] table of
the first-64 key indices per bucket, then gather per query via one-hot
matmuls. Sharding: 8 cores = 4 batches x 2 bucket-halves; host sums the
pair and subtracts 1 (tables store j+1, empty=0).

v2 layout/pipeline notes:
- k/q arrive packed [128, 1024]: partitions 0:64 carry D for j-chunks
  {0,2}-* no: cols 0:512 hold j 0:512 (parts 0:64) and j 512:1024
  (parts 64:128); cols 512:1024 hold j 1024:1536 / 1536:2048. One
  [128,512] binarize unlocks a full scan-half of hash matmuls.
- hash = two fp16 matmuls (W-hi + W-lo) accumulating f32 psum (sign
  margins ~1e-4; fp16 pair keeps representation error ~1e-6).
- k-side: relu half-0 split ACT||DVE to unblock the DVE scan sooner;
  scan half-1 runs on GPSIMD(Pool) chained off scan0's tail column.
- scatter idx via one copy_predicated into a preset -1 buffer
  (idx = 1-based rank at matches, -1 elsewhere).
- gather: 16 query-block matmuls x 2 half-tables accumulated in psum
  (no table merge); output DMA'd f32 straight from PSUM in quarters on
  parallel queues; host unpermutes the [128,16,64] block layout.
"""

import numpy as np
import ml_dtypes

B, L, D, NH = 4, 2048, 64, 8
KMAX = 64
TABLE_ELEMS = 1026  # local_scatter table size; 1-based ranks, slot 0 unused
MPAD = 40           # hash matmul lhsT free size: 8 real + 32 zero rows
HALF = L // 2
NBLK = 16           # query blocks of 128

_cache = {}


def _build_program():
    import concourse.bass as bass
    import concourse.mybir as mybir
    from concourse import bacc, tile
    from contextlib import ExitStack

    dt = mybir.dt
    Alu = mybir.AluOpType
    Act = mybir.ActivationFunctionType

    nc = bacc.Bacc("TRN2", target_bir_lowering=False, debug=False)

    # DRAM I/O (per-core shapes)
    kT_d = nc.declare_dram_parameter("kTp", [128, HALF], dt.bfloat16, isOutput=False)
    qT_d = nc.declare_dram_parameter("qTp", [128, HALF], dt.bfloat16, isOutput=False)
    wpk_d = nc.declare_dram_parameter("wpk", [128, 2 * MPAD], dt.float16, isOutput=False)
    sgnc_d = nc.declare_dram_parameter("sgnc", [128, 128], dt.float16, isOutput=False)
    out_d = nc.declare_dram_parameter("out", [128, NBLK * KMAX], dt.float16, isOutput=True)

    with ExitStack() as ctx:
        tc = ctx.enter_context(tile.TileContext(nc))
        sb = ctx.enter_context(tc.tile_pool(name="sb", bufs=1))
        hp = ctx.enter_context(tc.tile_pool(name="hp", bufs=2, space="PSUM"))
        aps = ctx.enter_context(tc.tile_pool(name="aps", bufs=2, space="PSUM"))
        gp = ctx.enter_context(tc.tile_pool(name="gp", bufs=2, space="PSUM"))

        from concourse.tile_rust import add_dep_helper  # noqa: E402

        # ---- input DMAs: ALL on the sync queue (data is consumable at
        # slice-end; other queues' completion sems land ~1.8us later).
        # Serial 500ns slices: kTa, wpk, sgnc, kTb, qTa, qTb. ----
        kT_sb = sb.tile([128, HALF], dt.bfloat16, tag="kT")
        qT_sb = sb.tile([128, HALF], dt.bfloat16, tag="qT")
        wpk_sb = sb.tile([128, 2 * MPAD], dt.float16, tag="wpk")
        sgnc_sb = sb.tile([128, 128], dt.float16, tag="sgnc")
        nc.sync.dma_start(kT_sb[:, 0:512], kT_d[:, 0:512])
        nc.sync.dma_start(wpk_sb[:], wpk_d[:])
        nc.sync.dma_start(sgnc_sb[:], sgnc_d[:])
        nc.sync.dma_start(kT_sb[:, 512:1024], kT_d[:, 512:1024])
        nc.sync.dma_start(qT_sb[:, 0:512], qT_d[:, 0:512])
        nc.sync.dma_start(qT_sb[:, 512:1024], qT_d[:, 512:1024])

        # scatter values: each partition holds 1..L (fp16; ints <= 2048 exact)
        iota_sb = sb.tile([128, L], dt.float16, tag="iota")
        nc.gpsimd.iota(iota_sb[:], pattern=[[1, L]], base=1, channel_multiplier=0,
                       allow_small_or_imprecise_dtypes=True)

        # bias constant for Relu(agree - 7)
        bias7 = sb.tile([128, 1], dt.float32, tag="bias7")
        nc.gpsimd.memset(bias7[:], -7.0)

        # ---- binarize (DVE, {0,1} fp16) ----
        xk = sb.tile([128, HALF], dt.float16, tag="xk")
        xq = sb.tile([128, HALF], dt.float16, tag="xq")
        bin_ops = {}
        for tag, src, dst, a, b in (
            ("kA", kT_sb, xk, 0, 512),
            ("qA", qT_sb, xq, 0, 512),
            ("kB", kT_sb, xk, 512, 1024),
            ("qB", qT_sb, xq, 512, 1024),
        ):
            bin_ops[tag] = nc.vector.tensor_single_scalar(
                dst[:, a:b], src[:, a:b], 0.0, Alu.is_gt
            )

        # ---- k-side: hash -> sign -> agree -> onehot, per j-half ----
        # pair h covers j-chunks (2h, 2h+1): rhs partitions 0:64 / 64:128 of
        # packed cols 512h:512h+512. psum 2-stack at row bases {0 (M=40 pad),
        # 32 (M=8)}.
        oh_k = sb.tile([128, L], dt.float16, tag="ohk")
        rank = sb.tile([128, L], dt.float16, tag="rank")
        sk = []
        agree_k = []
        relu_insts = []
        for h in range(2):
            c0, c1 = 512 * h, 512 * (h + 1)
            t = hp.tile([128, 512], dt.float32, tag="hp")
            nc.tensor.matmul(t[0:MPAD, :], lhsT=wpk_sb[0:64, 0:MPAD],
                             rhs=xk[0:64, c0:c1], start=True, stop=False)
            nc.tensor.matmul(t[0:MPAD, :], lhsT=wpk_sb[0:64, MPAD:2 * MPAD],
                             rhs=xk[0:64, c0:c1], start=False, stop=True)
            nc.tensor.matmul(t[32:40, :], lhsT=wpk_sb[64:128, 0:NH],
                             rhs=xk[64:128, c0:c1], start=True, stop=False)
            nc.tensor.matmul(t[32:40, :], lhsT=wpk_sb[64:128, MPAD:MPAD + NH],
                             rhs=xk[64:128, c0:c1], start=False, stop=True)
            s = sb.tile([MPAD, 512], dt.float16, tag=f"sk{h}")
            nc.scalar.activation(s[:], t[0:MPAD, :], Act.Sign)
            sk.append(s)
            apt = aps.tile([128, 1024], dt.float32, tag="agree")
            agree_k.append(
                nc.tensor.matmul(apt[:, 0:512], lhsT=sgnc_sb[0:8, :],
                                 rhs=s[0:8, :], start=True, stop=True)
            )
            agree_k.append(
                nc.tensor.matmul(apt[:, 512:1024], lhsT=sgnc_sb[32:40, :],
                                 rhs=s[32:40, :], start=True, stop=True)
            )
            lo = 1024 * h
            if h == 0:
                # split relu: ACT does the first 512 cols, DVE the second,
                # so scan half-0 starts as early as possible
                relu_insts.append(nc.scalar.activation(
                    oh_k[:, lo:lo + 512], apt[:, 0:512],
                    Act.Relu, bias=bias7[:]))
                nc.vector.tensor_single_scalar(
                    oh_k[:, lo + 512:lo + 1024], apt[:, 512:1024], 7.0, Alu.is_gt
                )
            else:
                relu_insts.append(nc.scalar.activation(
                    oh_k[:, lo:lo + 1024], apt[:],
                    Act.Relu, bias=bias7[:]))

        # ---- rank scan (DVE) + idx mask + scatter, pipelined per half:
        # scan0, m1/idx0 (scatter0 starts), scan1 (chained), m1/idx1 ----
        m1 = sb.tile([128, L], dt.float16, tag="m1")
        idx16 = sb.tile([128, L], dt.int16, tag="idx16")
        tabs = []
        idx_insts = []
        scan_insts = []
        for h in range(2):
            lo, hi = HALF * h, HALF * (h + 1)
            init = 0.0 if h == 0 else rank[:, HALF - 1:HALF]
            sc_i = nc.vector.tensor_tensor_scan(
                rank[:, lo:hi], oh_k[:, lo:hi], oh_k[:, lo:hi],
                init, Alu.add, Alu.bypass,
            )
            scan_insts.append(sc_i)
            if h == 1:
                # DVE priority: finish half-0 masks before scan half-1
                add_dep_helper(sc_i.ins, idx_insts[0].ins, sync=False,
                               reason="idx0 before scan1 on DVE")
            nc.vector.tensor_mul(m1[:, lo:hi], oh_k[:, lo:hi], rank[:, lo:hi])
            idx_insts.append(nc.vector.tensor_single_scalar(
                idx16[:, lo:hi], m1[:, lo:hi], 1.0, Alu.subtract
            ))
            tab = sb.tile([128, TABLE_ELEMS], dt.float16, tag=f"table{h}")
            tabs.append(tab)
            nc.gpsimd.local_scatter(
                tab[:], iota_sb[:, lo:hi], idx16[:, lo:hi],
                channels=128, num_elems=TABLE_ELEMS, num_idxs=HALF,
            )

        # ---- q-side: hash 2-stack pairs -> sign -> agree -> relu 3-way ----
        q1h = sb.tile([128, L], dt.float16, tag="q1h")
        aq = []
        for h in range(2):
            c0, c1 = 512 * h, 512 * (h + 1)
            t = hp.tile([128, 512], dt.float32, tag="hp")
            mm = nc.tensor.matmul(t[0:MPAD, :], lhsT=wpk_sb[0:64, 0:MPAD],
                                  rhs=xq[0:64, c0:c1], start=True, stop=False)
            if h == 0:
                # PE priority: k-side agree matmuls (feeding relu_k1) must
                # precede the q hash stream
                add_dep_helper(mm.ins, agree_k[-1].ins, sync=False,
                               reason="k agree before q hash on PE")
            nc.tensor.matmul(t[0:MPAD, :], lhsT=wpk_sb[0:64, MPAD:2 * MPAD],
                             rhs=xq[0:64, c0:c1], start=False, stop=True)
            nc.tensor.matmul(t[32:40, :], lhsT=wpk_sb[64:128, 0:NH],
                             rhs=xq[64:128, c0:c1], start=True, stop=False)
            nc.tensor.matmul(t[32:40, :], lhsT=wpk_sb[64:128, MPAD:MPAD + NH],
                             rhs=xq[64:128, c0:c1], start=False, stop=True)
            s = sb.tile([MPAD, 512], dt.float16, tag=f"sq{h}")
            sg = nc.scalar.activation(s[:], t[0:MPAD, :], Act.Sign)
            if h == 0:
                # ACT priority: k relus before q signs
                add_dep_helper(sg.ins, relu_insts[1].ins, sync=False,
                               reason="k relu before q sign on ACT")
            apt = aps.tile([128, 1024], dt.float32, tag="agree")
            aq.append(apt)
            nc.tensor.matmul(apt[:, 0:512], lhsT=sgnc_sb[0:8, :],
                             rhs=s[0:8, :], start=True, stop=True)
            nc.tensor.matmul(apt[:, 512:1024], lhsT=sgnc_sb[32:40, :],
                             rhs=s[32:40, :], start=True, stop=True)
        # relu_q 3-way: ACT [0:1024], DVE [1024:1536], ACT [1536:2048]
        nc.scalar.activation(q1h[:, 0:1024], aq[0][:], Act.Relu, bias=bias7[:])
        rqb1 = nc.vector.tensor_single_scalar(q1h[:, 1024:1536], aq[1][:, 0:512],
                                              7.0, Alu.is_gt)
        add_dep_helper(rqb1.ins, idx_insts[1].ins, sync=False,
                       reason="k idx chain before q relu on DVE")
        nc.scalar.activation(q1h[:, 1536:2048], aq[1][:, 512:1024],
                             Act.Relu, bias=bias7[:])

        # ---- gather: out[128t+p, s] = sum_c q1h[c,128t+p] * tab[c, 1+s];
        # two half-tables accumulate in psum (no merge op) ----
        gps = []
        for gi in range(2):
            gpt = gp.tile([128, 8 * KMAX], dt.float32, tag="gp", name=f"gp{gi}")
            gps.append(gpt)
        for blk in range(NBLK):
            op = gps[blk // 8]
            cc = KMAX * (blk % 8)
            nc.tensor.matmul(op[:, cc:cc + KMAX],
                             lhsT=q1h[:, 128 * blk:128 * (blk + 1)],
                             rhs=tabs[0][:, 0:KMAX], start=True, stop=False)
            nc.tensor.matmul(op[:, cc:cc + KMAX],
                             lhsT=q1h[:, 128 * blk:128 * (blk + 1)],
                             rhs=tabs[1][:, 0:KMAX], start=False, stop=True)

        # ---- out: psum -> fp16 SBUF halves (ACT || DVE), then half-DMAs
        # on the sync and scalar queues in parallel ----
        HW = 8 * KMAX
        out_sb = sb.tile([128, NBLK * KMAX], dt.float16, tag="out_sb")
        nc.scalar.activation(out_sb[:, 0:HW], gps[0][:], Act.Copy)
        nc.vector.tensor_copy(out_sb[:, HW:2 * HW], gps[1][:])
        nc.sync.dma_start(out_d[:, 0:HW], out_sb[:, 0:HW])
        nc.scalar.dma_start(out_d[:, HW:2 * HW], out_sb[:, HW:2 * HW])

    nc.compile()
    return nc


def _get_nc():
    if "nc" not in _cache:
        _cache["nc"] = _build_program()
    return _cache["nc"]


def _pack(xT):
    # [D, L] -> [128, L/2]: chunk c (512 cols) of j goes to
    # (partitions 64*(c%2) .., cols 512*(c//2) ..)
    x4 = xT.reshape(D, 4, 512)
    out = np.empty((128, HALF), xT.dtype)
    out[0:64, 0:512] = x4[:, 0]
    out[64:128, 0:512] = x4[:, 1]
    out[0:64, 512:1024] = x4[:, 2]
    out[64:128, 512:1024] = x4[:, 3]
    return out


def _make_in_maps(query, key, W):
    query = np.asarray(query, dtype=np.float32)
    key = np.asarray(key, dtype=np.float32)
    W = np.asarray(W, dtype=np.float32)
    qTp = [
        _pack(np.ascontiguousarray(query[b].T).astype(ml_dtypes.bfloat16))
        for b in range(B)
    ]
    kTp = [
        _pack(np.ascontiguousarray(key[b].T).astype(ml_dtypes.bfloat16))
        for b in range(B)
    ]

    wpk = np.zeros((128, 2 * MPAD), np.float16)
    wpk[0:64, :NH] = W.astype(np.float16)
    wpk[0:64, MPAD:MPAD + NH] = (W - wpk[0:64, :NH].astype(np.float32)).astype(
        np.float16
    )
    wpk[64:128] = wpk[0:64]

    sgnc = []
    for h in range(2):
        cg = 128 * h + np.arange(128)  # global bucket ids of this half
        bits = ((cg[None, :] >> np.arange(NH)[:, None]) & 1).astype(np.float32)
        pm = (2.0 * bits - 1.0).astype(np.float16)  # [8, 128]
        arr = np.zeros((128, 128), np.float16)
        for base in (0, 32, 64, 96):
            arr[base:base + NH] = pm
        sgnc.append(arr)
    return [
        {"qTp": qTp[c // 2], "kTp": kTp[c // 2], "wpk": wpk, "sgnc": sgnc[c % 2]}
        for c in range(2 * B)
    ]


def _combine(results):
    out = np.empty((B, L, KMAX), dtype=np.int64)
    for b in range(B):
        g = np.zeros((L, KMAX), np.float64)
        for h in range(2):
            r = results[2 * b + h]["out"].astype(np.float64)
            g += r.reshape(128, NBLK, KMAX).transpose(1, 0, 2).reshape(L, KMAX)
        out[b] = (g - 1.0).astype(np.int64)
    return out


def _run_spmd(in_maps, **kwargs):
    from concourse.bass_utils import run_bass_kernel_spmd

    return run_bass_kernel_spmd(_get_nc(), in_maps, list(range(2 * B)), **kwargs)


def kernel(query, key, W, head_idx=0, **_unused):
    in_maps = _make_in_maps(query, key, W)
    res = _run_spmd(in_maps)
    return _combine(res.results)


# revision 27
# speedup vs baseline: 1.0880x; 1.0271x over previous
"""Trainium2 Bass kernel for nn_CandidateFinder (LSH hash-equality KNN).

Reference semantics: q/k binarized (x>0), projected by W [64,8], sign bits
packed into an 8-bit bucket code; for each query, return the first 64 key
indices (ascending) whose code equals the query's code, padded with -1.

Key insight: codes live in [0,256). Build, per batch, a [256, 64+] table of
the first-64 key indices per bucket, then gather per query via one-hot
matmuls. Sharding: 8 cores = 4 batches x 2 bucket-halves; host sums the
pair and subtracts 1 (tables store j+1, empty=0).

v2 layout/pipeline notes:
- k/q arrive packed [128, 1024]: partitions 0:64 carry D for j-chunks
  {0,2}-* no: cols 0:512 hold j 0:512 (parts 0:64) and j 512:1024
  (parts 64:128); cols 512:1024 hold j 1024:1536 / 1536:2048. One
  [128,512] binarize unlocks a full scan-half of hash matmuls.
- hash = two fp16 matmuls (W-hi + W-lo) accumulating f32 psum (sign
  margins ~1e-4; fp16 pair keeps representation error ~1e-6).
- k-side: relu half-0 split ACT||DVE to unblock the DVE scan sooner;
  scan half-1 runs on GPSIMD(Pool) chained off scan0's tail column.
- scatter idx via one copy_predicated into a preset -1 buffer
  (idx = 1-based rank at matches, -1 elsewhere).
- gather: 16 query-block matmuls x 2 half-tables accumulated in psum
  (no table merge); output DMA'd f32 straight from PSUM in quarters on
  parallel queues; host unpermutes the [128,16,64] block layout.
"""

import numpy as np
import ml_dtypes

B, L, D, NH = 4, 2048, 64, 8
KMAX = 64
TABLE_ELEMS = 1026  # local_scatter table size; 1-based ranks, slot 0 unused
MPAD = 40           # hash matmul lhsT free size: 8 real + 32 zero rows
HALF = L // 2
NBLK = 16           # query blocks of 128

_cache = {}


def _build_program():
    import concourse.bass as bass
    import concourse.mybir as mybir
    from concourse import bacc, tile
    from contextlib import ExitStack

    dt = mybir.dt
    Alu = mybir.AluOpType
    Act = mybir.ActivationFunctionType

    nc = bacc.Bacc("TRN2", target_bir_lowering=False, debug=False)

    # DRAM I/O (per-core shapes)
    kT_d = nc.declare_dram_parameter("kTp", [128, HALF], dt.bfloat16, isOutput=False)
    qT_d = nc.declare_dram_parameter("qTp", [128, HALF], dt.bfloat16, isOutput=False)
    wpk_d = nc.declare_dram_parameter("wpk", [128, 2 * MPAD], dt.float16, isOutput=False)
    sgnc_d = nc.declare_dram_parameter("sgnc", [128, 128], dt.float16, isOutput=False)
    out_d = nc.declare_dram_parameter("out", [128, NBLK * KMAX], dt.float16, isOutput=True)

    with ExitStack() as ctx:
        tc = ctx.enter_context(tile.TileContext(nc))
        sb = ctx.enter_context(tc.tile_pool(name="sb", bufs=1))
        hp = ctx.enter_context(tc.tile_pool(name="hp", bufs=2, space="PSUM"))
        aps = ctx.enter_context(tc.tile_pool(name="aps", bufs=2, space="PSUM"))
        gp = ctx.enter_context(tc.tile_pool(name="gp", bufs=2, space="PSUM"))

        from concourse.tile_rust import add_dep_helper  # noqa: E402

        # ---- input DMAs: k/q on the sync queue (completion train: first at
        # slice-end+1717, then ~500 apart); consts on the gpsimd queue in
        # parallel so the weights don't push the train out. ----
        kT_sb = sb.tile([128, HALF], dt.bfloat16, tag="kT")
        qT_sb = sb.tile([128, HALF], dt.bfloat16, tag="qT")
        wpk_sb = sb.tile([128, 2 * MPAD], dt.float16, tag="wpk")
        sgnc_sb = sb.tile([128, 128], dt.float16, tag="sgnc")
        nc.sync.dma_start(kT_sb[:, 0:512], kT_d[:, 0:512])
        nc.sync.dma_start(kT_sb[:, 512:1024], kT_d[:, 512:1024])
        nc.sync.dma_start(qT_sb[:, 0:512], qT_d[:, 0:512])
        nc.sync.dma_start(qT_sb[:, 512:1024], qT_d[:, 512:1024])
        nc.gpsimd.dma_start(wpk_sb[:], wpk_d[:])
        nc.gpsimd.dma_start(sgnc_sb[:], sgnc_d[:])

        # scatter values: each partition holds 1..L (fp16; ints <= 2048 exact)
        iota_sb = sb.tile([128, L], dt.float16, tag="iota")
        nc.gpsimd.iota(iota_sb[:], pattern=[[1, L]], base=1, channel_multiplier=0,
                       allow_small_or_imprecise_dtypes=True)

        # bias constant for Relu(agree - 7)
        bias7 = sb.tile([128, 1], dt.float32, tag="bias7")
        nc.gpsimd.memset(bias7[:], -7.0)

        # ---- binarize (DVE, {0,1} fp16) ----
        xk = sb.tile([128, HALF], dt.float16, tag="xk")
        xq = sb.tile([128, HALF], dt.float16, tag="xq")
        bin_ops = {}
        for tag, src, dst, a, b in (
            ("kA", kT_sb, xk, 0, 512),
            ("qA", qT_sb, xq, 0, 512),
            ("kB", kT_sb, xk, 512, 1024),
            ("qB", qT_sb, xq, 512, 1024),
        ):
            bin_ops[tag] = nc.vector.tensor_single_scalar(
                dst[:, a:b], src[:, a:b], 0.0, Alu.is_gt
            )

        # ---- k-side: hash -> sign -> agree -> onehot, per j-half ----
        # pair h covers j-chunks (2h, 2h+1): rhs partitions 0:64 / 64:128 of
        # packed cols 512h:512h+512. psum 2-stack at row bases {0 (M=40 pad),
        # 32 (M=8)}.
        oh_k = sb.tile([128, L], dt.float16, tag="ohk")
        rank = sb.tile([128, L], dt.float16, tag="rank")
        sk = []
        agree_k = []
        relu_insts = []
        for h in range(2):
            c0, c1 = 512 * h, 512 * (h + 1)
            t = hp.tile([128, 512], dt.float32, tag="hp")
            nc.tensor.matmul(t[0:MPAD, :], lhsT=wpk_sb[0:64, 0:MPAD],
                             rhs=xk[0:64, c0:c1], start=True, stop=False)
            nc.tensor.matmul(t[0:MPAD, :], lhsT=wpk_sb[0:64, MPAD:2 * MPAD],
                             rhs=xk[0:64, c0:c1], start=False, stop=True)
            nc.tensor.matmul(t[32:40, :], lhsT=wpk_sb[64:128, 0:NH],
                             rhs=xk[64:128, c0:c1], start=True, stop=False)
            nc.tensor.matmul(t[32:40, :], lhsT=wpk_sb[64:128, MPAD:MPAD + NH],
                             rhs=xk[64:128, c0:c1], start=False, stop=True)
            s = sb.tile([MPAD, 512], dt.float16, tag=f"sk{h}")
            nc.scalar.activation(s[:], t[0:MPAD, :], Act.Sign)
            sk.append(s)
            apt = aps.tile([128, 1024], dt.float32, tag="agree")
            agree_k.append(
                nc.tensor.matmul(apt[:, 0:512], lhsT=sgnc_sb[0:8, :],
                                 rhs=s[0:8, :], start=True, stop=True)
            )
            agree_k.append(
                nc.tensor.matmul(apt[:, 512:1024], lhsT=sgnc_sb[32:40, :],
                                 rhs=s[32:40, :], start=True, stop=True)
            )
            lo = 1024 * h
            relu_insts.append(nc.scalar.activation(
                oh_k[:, lo:lo + 1024], apt[:],
                Act.Relu, bias=bias7[:]))

        # ---- rank scan (DVE) + idx mask + scatter, pipelined per half:
        # scan0, m1/idx0 (scatter0 starts), scan1 (chained), m1/idx1 ----
        m1 = sb.tile([128, L], dt.float16, tag="m1")
        idx16 = sb.tile([128, L], dt.int16, tag="idx16")
        tabs = []
        idx_insts = []
        segs = [(0, 1024), (1024, 2048)]
        nc.vector.tensor_tensor_scan(
            rank[:, 0:HALF], oh_k[:, 0:HALF], oh_k[:, 0:HALF],
            0.0, Alu.add, Alu.bypass,
        )
        scan1 = nc.vector.tensor_tensor_scan(
            rank[:, HALF:L], oh_k[:, HALF:L], oh_k[:, HALF:L],
            rank[:, HALF - 1:HALF], Alu.add, Alu.bypass,
        )
        for si, (lo, hi) in enumerate(segs):
            nc.vector.tensor_mul(m1[:, lo:hi], oh_k[:, lo:hi], rank[:, lo:hi])
            idx_insts.append(nc.vector.tensor_single_scalar(
                idx16[:, lo:hi], m1[:, lo:hi], 1.0, Alu.subtract
            ))
            tab = sb.tile([128, TABLE_ELEMS], dt.float16, tag=f"table{si}")
            tabs.append(tab)
            nc.gpsimd.local_scatter(
                tab[:], iota_sb[:, lo:hi], idx16[:, lo:hi],
                channels=128, num_elems=TABLE_ELEMS, num_idxs=hi - lo,
            )
        # DVE priority: half-0 masks right after scan0, before scan1
        add_dep_helper(scan1.ins, idx_insts[0].ins, sync=False,
                       reason="idx seg0 before scan1 on DVE")

        # ---- q-side: hash 2-stack pairs -> sign -> agree -> relu 3-way ----
        q1h = sb.tile([128, L], dt.float16, tag="q1h")
        aq = []
        for h in range(2):
            c0, c1 = 512 * h, 512 * (h + 1)
            t = hp.tile([128, 512], dt.float32, tag="hp")
            mm = nc.tensor.matmul(t[0:MPAD, :], lhsT=wpk_sb[0:64, 0:MPAD],
                                  rhs=xq[0:64, c0:c1], start=True, stop=False)
            if h == 0:
                # PE priority: k-side agree matmuls (feeding relu_k1) must
                # precede the q hash stream
                add_dep_helper(mm.ins, agree_k[-1].ins, sync=False,
                               reason="k agree before q hash on PE")
            nc.tensor.matmul(t[0:MPAD, :], lhsT=wpk_sb[0:64, MPAD:2 * MPAD],
                             rhs=xq[0:64, c0:c1], start=False, stop=True)
            nc.tensor.matmul(t[32:40, :], lhsT=wpk_sb[64:128, 0:NH],
                             rhs=xq[64:128, c0:c1], start=True, stop=False)
            nc.tensor.matmul(t[32:40, :], lhsT=wpk_sb[64:128, MPAD:MPAD + NH],
                             rhs=xq[64:128, c0:c1], start=False, stop=True)
            s = sb.tile([MPAD, 512], dt.float16, tag=f"sq{h}")
            sg = nc.scalar.activation(s[:], t[0:MPAD, :], Act.Sign)
            if h == 0:
                # ACT priority: k relus before q signs
                add_dep_helper(sg.ins, relu_insts[1].ins, sync=False,
                               reason="k relu before q sign on ACT")
            apt = aps.tile([128, 1024], dt.float32, tag="agree")
            aq.append(apt)
            nc.tensor.matmul(apt[:, 0:512], lhsT=sgnc_sb[0:8, :],
                             rhs=s[0:8, :], start=True, stop=True)
            nc.tensor.matmul(apt[:, 512:1024], lhsT=sgnc_sb[32:40, :],
                             rhs=s[32:40, :], start=True, stop=True)
        # relu_q on ACT (DVE is saturated by the scan/mask chain)
        nc.scalar.activation(q1h[:, 0:1024], aq[0][:], Act.Relu, bias=bias7[:])
        nc.scalar.activation(q1h[:, 1024:2048], aq[1][:], Act.Relu, bias=bias7[:])

        # ---- gather: out[128t+p, s] = sum_c q1h[c,128t+p] * tab[c, s];
        # three segment-tables accumulate in psum (no merge op). Emission
        # order interleaves table segments so each gather wave starts as
        # soon as its scatter lands. ----
        gps = []
        for gi in range(2):
            gpt = gp.tile([128, 8 * KMAX], dt.float32, tag="gp", name=f"gp{gi}")
            gps.append(gpt)

        for blk in range(NBLK):
            op = gps[blk // 8]
            cc = KMAX * (blk % 8)
            nc.tensor.matmul(op[:, cc:cc + KMAX],
                             lhsT=q1h[:, 128 * blk:128 * (blk + 1)],
                             rhs=tabs[0][:, 0:KMAX], start=True, stop=False)
            nc.tensor.matmul(op[:, cc:cc + KMAX],
                             lhsT=q1h[:, 128 * blk:128 * (blk + 1)],
                             rhs=tabs[1][:, 0:KMAX], start=False, stop=True)

        # ---- out: psum -> fp16 SBUF halves (ACT || DVE, separate tiles to
        # avoid write-tracker serialization), half-DMAs on sync + scalar ----
        HW = 8 * KMAX
        outA = sb.tile([128, HW], dt.float16, tag="outA")
        outB = sb.tile([128, HW], dt.float16, tag="outB")
        nc.scalar.activation(outA[:], gps[0][:], Act.Copy)
        nc.vector.tensor_copy(outB[:], gps[1][:])
        nc.sync.dma_start(out_d[:, 0:HW], outA[:])
        nc.scalar.dma_start(out_d[:, HW:2 * HW], outB[:])

    nc.compile()
    return nc


def _get_nc():
    if "nc" not in _cache:
        _cache["nc"] = _build_program()
    return _cache["nc"]


def _pack(xT):
    # [D, L] -> [128, L/2]: chunk c (512 cols) of j goes to
    # (partitions 64*(c%2) .., cols 512*(c//2) ..)
    x4 = xT.reshape(D, 4, 512)
    out = np.empty((128, HALF), xT.dtype)
    out[0:64, 0:512] = x4[:, 0]
    out[64:128, 0:512] = x4[:, 1]
    out[0:64, 512:1024] = x4[:, 2]
    out[64:128, 512:1024] = x4[:, 3]
    return out


def _make_in_maps(query, key, W):
    query = np.asarray(query, dtype=np.float32)
    key = np.asarray(key, dtype=np.float32)
    W = np.asarray(W, dtype=np.float32)
    qTp = [
        _pack(np.ascontiguousarray(query[b].T).astype(ml_dtypes.bfloat16))
        for b in range(B)
    ]
    kTp = [
        _pack(np.ascontiguousarray(key[b].T).astype(ml_dtypes.bfloat16))
        for b in range(B)
    ]

    wpk = np.zeros((128, 2 * MPAD), np.float16)
    wpk[0:64, :NH] = W.astype(np.float16)
    wpk[0:64, MPAD:MPAD + NH] = (W - wpk[0:64, :NH].astype(np.float32)).astype(
        np.float16
    )
    wpk[64:128] = wpk[0:64]

    sgnc = []
    for h in range(2):
        cg = 128 * h + np.arange(128)  # global bucket ids of this half
        bits = ((cg[None, :] >> np.arange(NH)[:, None]) & 1).astype(np.float32)
        pm = (2.0 * bits - 1.0).astype(np.float16)  # [8, 128]
        arr = np.zeros((128, 128), np.float16)
        for base in (0, 32, 64, 96):
            arr[base:base + NH] = pm
        sgnc.append(arr)
    return [
        {"qTp": qTp[c // 2], "kTp": kTp[c // 2], "wpk": wpk, "sgnc": sgnc[c % 2]}
        for c in range(2 * B)
    ]


def _combine(results):
    out = np.empty((B, L, KMAX), dtype=np.int64)
    for b in range(B):
        g = np.zeros((L, KMAX), np.float64)
        for h in range(2):
            r = results[2 * b + h]["out"].astype(np.float64)
            g += r.reshape(128, NBLK, KMAX).transpose(1, 0, 2).reshape(L, KMAX)
        out[b] = (g - 1.0).astype(np.int64)
    return out


def _run_spmd(in_maps, **kwargs):
    from concourse.bass_utils import run_bass_kernel_spmd

    return run_bass_kernel_spmd(_get_nc(), in_maps, list(range(2 * B)), **kwargs)


def kernel(query, key, W, head_idx=0, **_unused):
    in_maps = _make_in_maps(query, key, W)
    res = _run_spmd(in_maps)
    return _combine(res.results)
